# revision 10
# baseline (speedup 1.0000x reference)
"""Binarized-MLP (BinaryNet) forward on 8 Trainium2 NeuronCores.

Reference computation (per nn_FC_large):
    h = sign(x[:, :768]) @ sign(w1).T + b1 ; BN1 ; -> sign
    h = sign(h) @ sign(w2).T + b2         ; BN2 ; -> sign
    h = sign(h) @ sign(w3).T + b3         ; BN3 ; -> sign
    h = sign(h) @ sign(w4).T + b4         ; BN4 ; log_softmax

Strategy (data parallel, batch 16384 -> 2048 rows/core):
  * All matmul operands are exactly representable in fp8: weights are
    binarized host-side to {-1,+1}; activations are kept as a in {0,1}
    (a = [pre-act >= 0]) and the identity
        sign_mm = 2*(Wsign @ a) - rowsum(Wsign)
    folds rowsum into per-neuron thresholds, so each layer's epilogue is a
    single DVE is_ge producing the next layer's {0,1} fp8 activations.
  * Matmuls run in fp8e4 with perf_mode=DoubleRow (K=256 per instruction),
    activations stored feature-major [F, B] in SBUF across the whole net.
  * BatchNorm (eval) + bias fold into thresholds (layers 1-3) / an affine
    (layer 4). Layer-4 logits are PE-transposed to batch-major and
    log_softmax runs on-device (DVE/ACT).
  * Accumulation is exact: products are in {-1,0,1}, sums are integers
    well inside fp32, so the binary pipeline is bit-exact w.r.t. the
    reference up to thresholds ties (probability ~0 with random BN stats).

Everything is hardcoded for x:[16384,784], layers 768->4096->4096->4096->10.
"""

import numpy as np
import ml_dtypes
from contextlib import ExitStack

import concourse.bass as bass
import concourse.mybir as mybir
import concourse.tile as tile
from concourse import bacc
from concourse.bass_utils import run_bass_kernel_spmd
from concourse.masks import make_identity

FP32 = mybir.dt.float32
FP8 = mybir.dt.float8e4
NP_FP8 = ml_dtypes.float8_e4m3

EPS = 1e-5
B, IND, HID, OUT = 16384, 768, 4096, 10
N_CORES = 8
BC = B // N_CORES  # 2048 batch rows per core

# Knobs (test.py may flip TRACE before calling kernel()).
TRACE = False
TRACE_KWARGS = {}
LAST_RESULTS = None  # BassKernelResults of the most recent run


# --------------------------------------------------------------------------
# Device program
# --------------------------------------------------------------------------

def _layer_fwd(nc, wpool, psum_pool, act_in, C, wdr, thr_sb, act_out, Mt, bc):
    """One binarized layer: act_out = [W_fp8dr.T @ act_in >= thr] in {0,1} fp8.

    act_in : SBUF AP [128, C, 2, bc] fp8 ({0,1})
    wdr    : DRAM [Mt, 128, C, 2, 128] fp8 ({-1,+1})
    thr_sb : SBUF [128, Mt] fp32
    act_out: SBUF AP [128, Mt//2, 2, bc] fp8
    """
    NT = bc // 512
    DR = mybir.MatmulPerfMode.DoubleRow
    for mt in range(Mt):
        wt = wpool.tile([128, C, 2, 128], FP8, tag="w")
        nc.sync.dma_start(out=wt[:], in_=wdr[mt])
        pss = [psum_pool.tile([128, 512], FP32, tag="psum", name=f"ps{mt}_{n}")
               for n in range(NT)]
        for c in range(C):
            for n in range(NT):
                nc.tensor.matmul(
                    pss[n][:],
                    lhsT=wt[:, c, :, :],
                    rhs=act_in[:, c, :, 512 * n:512 * (n + 1)],
                    start=(c == 0),
                    stop=(c == C - 1),
                    perf_mode=DR,
                )
        for n in range(NT):
            nc.vector.tensor_scalar(
                out=act_out[:, mt // 2, mt % 2, 512 * n:512 * (n + 1)],
                in0=pss[n][:],
                scalar1=thr_sb[:, mt:mt + 1],
                scalar2=None,
                op0=mybir.AluOpType.is_ge,
            )


def build_program(bc=BC, dump_acts=False):
    """Build the per-core Bass/Tile program (SPMD; identical on all cores)."""
    NT = bc // 512
    BT = bc // 128
    DR = mybir.MatmulPerfMode.DoubleRow

    nc = bacc.Bacc(None, target_bir_lowering=False, debug=False)
    dbg = {}
    if dump_acts:
        for nm in ("act1d", "act2d", "act3d", "act4d"):
            cdim = 3 if nm == "act1d" else 16
            dbg[nm] = nc.dram_tensor(
                nm, [128, cdim, 2, bc], FP8, kind="ExternalOutput")
        dbg["h4d"] = nc.dram_tensor("h4d", [16, bc], FP32, kind="ExternalOutput")

    xt = nc.dram_tensor("xt", [IND, bc], FP32, kind="ExternalInput")
    w1 = nc.dram_tensor("w1dr", [32, 128, 3, 2, 128], FP8, kind="ExternalInput")
    w2 = nc.dram_tensor("w2dr", [32, 128, 16, 2, 128], FP8, kind="ExternalInput")
    w3 = nc.dram_tensor("w3dr", [32, 128, 16, 2, 128], FP8, kind="ExternalInput")
    w4 = nc.dram_tensor("w4dr", [128, 16, 2, 16], FP8, kind="ExternalInput")
    thr1 = nc.dram_tensor("thr1", [128, 32], FP32, kind="ExternalInput")
    thr2 = nc.dram_tensor("thr2", [128, 32], FP32, kind="ExternalInput")
    thr3 = nc.dram_tensor("thr3", [128, 32], FP32, kind="ExternalInput")
    c4 = nc.dram_tensor("c4", [16, 2], FP32, kind="ExternalInput")
    out = nc.dram_tensor("out", [bc, OUT], FP32, kind="ExternalOutput")

    with tile.TileContext(nc) as tc, ExitStack() as ctx:
        consts = ctx.enter_context(tc.tile_pool(name="consts", bufs=1))
        xpool = ctx.enter_context(tc.tile_pool(name="xpool", bufs=2))
        a1pool = ctx.enter_context(tc.tile_pool(name="a1pool", bufs=1))
        apool = ctx.enter_context(
            tc.tile_pool(name="apool", bufs=3 if dump_acts else 2))
        wpool = ctx.enter_context(tc.tile_pool(name="wpool", bufs=3))
        smpool = ctx.enter_context(tc.tile_pool(name="smpool", bufs=3))
        psum_pool = ctx.enter_context(
            tc.tile_pool(name="psum", bufs=8, space="PSUM"))

        thr1_sb = consts.tile([128, 32], FP32, tag="thr1")
        thr2_sb = consts.tile([128, 32], FP32, tag="thr2")
        thr3_sb = consts.tile([128, 32], FP32, tag="thr3")
        c4_sb = consts.tile([16, 2], FP32, tag="c4")
        w4_sb = consts.tile([128, 16, 2, 16], FP8, tag="w4")
        ident = consts.tile([128, 128], FP32, tag="ident")
        h4 = consts.tile([16, bc], FP32, tag="h4")
        out_sb = consts.tile([128, BT, OUT], FP32, tag="outsb")

        nc.sync.dma_start(out=thr1_sb[:], in_=thr1[:])
        nc.sync.dma_start(out=thr2_sb[:], in_=thr2[:])
        nc.sync.dma_start(out=thr3_sb[:], in_=thr3[:])
        nc.sync.dma_start(out=c4_sb[:], in_=c4[:])
        nc.sync.dma_start(out=w4_sb[:], in_=w4[:])
        make_identity(nc, ident[:])

        # ---- binarize x: act1 = [x >= 0] in {0,1} fp8, feature-major ----
        act1 = a1pool.tile([128, 3, 2, bc], FP8, tag="act1")
        for i in range(6):
            xt_t = xpool.tile([128, bc], FP32, tag="x")
            nc.sync.dma_start(out=xt_t[:], in_=xt[128 * i:128 * (i + 1), :])
            nc.vector.tensor_scalar(
                out=act1[:, i // 2, i % 2, :],
                in0=xt_t[:],
                scalar1=0.0,
                scalar2=None,
                op0=mybir.AluOpType.is_ge,
            )

        # ---- layers 1-3 ----
        act2 = apool.tile([128, 16, 2, bc], FP8, tag="actbig")
        _layer_fwd(nc, wpool, psum_pool, act1, 3, w1, thr1_sb, act2, 32, bc)
        act3 = apool.tile([128, 16, 2, bc], FP8, tag="actbig")
        _layer_fwd(nc, wpool, psum_pool, act2, 16, w2, thr2_sb, act3, 32, bc)
        act4 = apool.tile([128, 16, 2, bc], FP8, tag="actbig")
        _layer_fwd(nc, wpool, psum_pool, act3, 16, w3, thr3_sb, act4, 32, bc)

        # ---- layer 4: logits (M padded 10->16), affine folds BN+rowsum ----
        for n in range(NT):
            ps4 = psum_pool.tile([16, 512], FP32, tag="psum")
            for c in range(16):
                nc.tensor.matmul(
                    ps4[:],
                    lhsT=w4_sb[:, c, :, :],
                    rhs=act4[:, c, :, 512 * n:512 * (n + 1)],
                    start=(c == 0),
                    stop=(c == 15),
                    perf_mode=DR,
                )
            nc.vector.tensor_scalar(
                out=h4[:, 512 * n:512 * (n + 1)],
                in0=ps4[:],
                scalar1=c4_sb[:, 0:1],
                scalar2=c4_sb[:, 1:2],
                op0=mybir.AluOpType.mult,
                op1=mybir.AluOpType.add,
            )

        # ---- transpose to batch-major + log_softmax over the 10 classes ----
        for bt in range(BT):
            tp = psum_pool.tile([128, OUT], FP32, tag="psum")
            nc.tensor.transpose(
                tp[:], h4[0:OUT, 128 * bt:128 * (bt + 1)], ident[0:OUT, 0:OUT])
            mx = smpool.tile([128, 1], FP32, tag="mx")
            nc.vector.reduce_max(mx[:], tp[:], axis=mybir.AxisListType.X)
            sh = smpool.tile([128, OUT], FP32, tag="sh")
            nc.vector.tensor_scalar(
                out=sh[:], in0=tp[:], scalar1=mx[:], scalar2=None,
                op0=mybir.AluOpType.subtract)
            ex = smpool.tile([128, OUT], FP32, tag="ex")
            se = smpool.tile([128, 1], FP32, tag="se")
            nc.scalar.activation(
                ex[:], sh[:], mybir.ActivationFunctionType.Exp,
                accum_out=se[:])
            ls = smpool.tile([128, 1], FP32, tag="ls")
            nc.scalar.activation(
                ls[:], se[:], mybir.ActivationFunctionType.Ln)
            nc.vector.tensor_scalar(
                out=out_sb[:, bt, :], in0=sh[:], scalar1=ls[:], scalar2=None,
                op0=mybir.AluOpType.subtract)

        nc.sync.dma_start(
            out=out.rearrange("(bt p) o -> p bt o", p=128), in_=out_sb[:])

        if dump_acts:
            nc.sync.dma_start(out=dbg["act1d"][:], in_=act1[:])
            nc.sync.dma_start(out=dbg["act2d"][:], in_=act2[:])
            nc.sync.dma_start(out=dbg["act3d"][:], in_=act3[:])
            nc.sync.dma_start(out=dbg["act4d"][:], in_=act4[:])
            nc.sync.dma_start(out=dbg["h4d"][:], in_=h4[:])

    nc.compile()
    return nc


# --------------------------------------------------------------------------
# Host-side preparation
# --------------------------------------------------------------------------

def _pack_w_dr(ws_t):
    """[Fin, Fout] {-1,+1} -> [Mt, 128, C, 2, 128] fp8 DoubleRow layout.

    wdr[mt, ki, c, ko, mi] = ws_t[256*c + 128*ko + ki, 128*mt + mi]
    """
    fin, fout = ws_t.shape
    C, Mt = fin // 256, fout // 128
    w = ws_t.reshape(C, 2, 128, Mt, 128).transpose(3, 2, 0, 1, 4)
    return np.ascontiguousarray(w).astype(NP_FP8)


def prepare_consts(inputs):
    """Fold sign(w), BN, bias and the 0/1-activation rowsum correction."""
    consts = {}
    for i in (1, 2, 3, 4):
        w = np.asarray(inputs[f"w{i}"]).astype(np.float64)
        b = np.asarray(inputs[f"b{i}"]).astype(np.float64)
        g = np.asarray(inputs[f"g{i}"]).astype(np.float64)
        be = np.asarray(inputs[f"be{i}"]).astype(np.float64)
        m = np.asarray(inputs[f"m{i}"]).astype(np.float64)
        v = np.asarray(inputs[f"v{i}"]).astype(np.float64)
        ws = np.where(w >= 0, 1.0, -1.0)          # [fo, fi]
        rowsum = ws.sum(axis=1)                   # [fo]
        alpha = g / np.sqrt(v + EPS)
        if i < 4:
            assert (alpha > 0).all(), "BN scale must be positive for is_ge fold"
            # a_next = 1  iff  BN(2*mmA - rowsum + b) >= 0  iff  mmA >= thr.
            # mmA is integer-valued, so mmA >= thr  <=>  mmA >= ceil(thr);
            # ceil(thr) is exact in fp32, making the device compare tie-free.
            thr = np.ceil((m - b - be / alpha + rowsum) / 2.0)
            consts[f"thr{i}"] = np.ascontiguousarray(
                thr.reshape(32, 128).T).astype(np.float32)
            consts[f"w{i}dr"] = _pack_w_dr(ws.T)
        else:
            # logits = mmA*(2*alpha) + ((b - m - rowsum)*alpha + be), pad to 16
            scale = 2.0 * alpha
            beta = (b - m - rowsum) * alpha + be
            c4 = np.zeros((16, 2), np.float32)
            c4[:10, 0] = scale.astype(np.float32)
            c4[:10, 1] = beta.astype(np.float32)
            consts["c4"] = c4
            ws_t_pad = np.zeros((HID, 16), np.float64)
            ws_t_pad[:, :10] = ws.T
            # w4dr[ki, c, ko, m] = ws_t_pad[256*c + 128*ko + ki, m]
            w4 = ws_t_pad.reshape(16, 2, 128, 16).transpose(2, 0, 1, 3)
            consts["w4dr"] = np.ascontiguousarray(w4).astype(NP_FP8)
    return consts


_PROG_CACHE = {}


def _get_program(bc=BC):
    if bc not in _PROG_CACHE:
        _PROG_CACHE[bc] = build_program(bc)
    return _PROG_CACHE[bc]


def kernel(**inputs):
    global LAST_RESULTS
    x = np.asarray(inputs["x"], np.float32)
    assert x.shape == (B, 784)
    consts = prepare_consts(inputs)
    xt_full = np.ascontiguousarray(x[:, :IND].T)  # [768, 16384]

    nc = _get_program(BC)
    in_maps = []
    for c in range(N_CORES):
        m = {"xt": np.ascontiguousarray(xt_full[:, c * BC:(c + 1) * BC])}
        m.update(consts)
        in_maps.append(m)

    res = run_bass_kernel_spmd(
        nc, in_maps, core_ids=list(range(N_CORES)), trace=TRACE,
        **TRACE_KWARGS)
    LAST_RESULTS = res
    return np.concatenate([r["out"] for r in res.results], axis=0)


# revision 20
# speedup vs baseline: 1.0424x; 1.0424x over previous
"""Binarized-MLP (BinaryNet) forward on 8 Trainium2 NeuronCores.

Reference computation (per nn_FC_large):
    h = sign(x[:, :768]) @ sign(w1).T + b1 ; BN1 ; -> sign
    h = sign(h) @ sign(w2).T + b2         ; BN2 ; -> sign
    h = sign(h) @ sign(w3).T + b3         ; BN3 ; -> sign
    h = sign(h) @ sign(w4).T + b4         ; BN4 ; log_softmax

Strategy (data parallel, batch 16384 -> 2048 rows/core):
  * All matmul operands are exactly representable in fp8: weights are
    binarized host-side to {-1,+1}; activations are kept as a in {0,1}
    (a = [pre-act >= 0]) and the identity
        sign_mm = 2*(Wsign @ a) - rowsum(Wsign)
    folds rowsum into per-neuron thresholds, so each layer's epilogue is a
    single DVE is_ge producing the next layer's {0,1} fp8 activations.
  * Matmuls run in fp8e4 with perf_mode=DoubleRow (K=256 per instruction),
    activations stored feature-major [F, B] in SBUF across the whole net.
  * BatchNorm (eval) + bias fold into thresholds (layers 1-3) / an affine
    (layer 4). Layer-4 logits are PE-transposed to batch-major and
    log_softmax runs on-device (DVE/ACT).
  * Accumulation is exact: products are in {-1,0,1}, sums are integers
    well inside fp32, so the binary pipeline is bit-exact w.r.t. the
    reference up to thresholds ties (probability ~0 with random BN stats).

Everything is hardcoded for x:[16384,784], layers 768->4096->4096->4096->10.
"""

import numpy as np
import ml_dtypes
from contextlib import ExitStack

import concourse.mybir as mybir
import concourse.tile as tile
from concourse import bacc
from concourse.bass_utils import run_bass_kernel_spmd
from concourse.masks import make_identity

FP32 = mybir.dt.float32
BF16 = mybir.dt.bfloat16
FP8 = mybir.dt.float8e4
NP_FP8 = ml_dtypes.float8_e4m3
NP_BF16 = ml_dtypes.bfloat16

EPS = 1e-5
B, IND, HID, OUT = 16384, 768, 4096, 10
N_CORES = 8
BC = B // N_CORES  # 2048 batch rows per core

# Knobs (test.py may flip TRACE before calling kernel()).
TRACE = False
TRACE_KWARGS = {}
LAST_RESULTS = None  # BassKernelResults of the most recent run


# --------------------------------------------------------------------------
# Device program
# --------------------------------------------------------------------------

def _layer_fwd(nc, wpool, psum_pool, act_in, C, wdr, thr_sb, act_out, Mt, bc,
               dma_engine=None):
    """One binarized layer: act_out = [W_fp8dr.T @ act_in >= thr] in {0,1} fp8.

    act_in : SBUF AP [128, C, 2, bc] fp8 ({0,1})
    wdr    : DRAM [Mt, 128, C, 2, 128] fp8 ({-1,+1})
    thr_sb : SBUF [128, Mt] fp32
    act_out: SBUF AP [128, Mt//2, 2, bc] fp8
    """
    NT = bc // 512
    DR = mybir.MatmulPerfMode.DoubleRow
    dma_engine = dma_engine or nc.sync
    for mt in range(Mt):
        wt = wpool.tile([128, C, 2, 128], FP8, tag="w")
        dma_engine.dma_start(out=wt[:], in_=wdr[mt])
        pss = [psum_pool.tile([128, 512], FP32, tag="psum", name=f"ps{mt}_{n}")
               for n in range(NT)]
        for c in range(C):
            for n in range(NT):
                nc.tensor.matmul(
                    pss[n][:],
                    lhsT=wt[:, c, :, :],
                    rhs=act_in[:, c, :, 512 * n:512 * (n + 1)],
                    start=(c == 0),
                    stop=(c == C - 1),
                    perf_mode=DR,
                )
        for n in range(NT):
            nc.vector.tensor_scalar(
                out=act_out[:, mt // 2, mt % 2, 512 * n:512 * (n + 1)],
                in0=pss[n][:],
                scalar1=thr_sb[:, mt:mt + 1],
                scalar2=None,
                op0=mybir.AluOpType.is_ge,
            )


def build_program(bc=BC, dump_acts=False):
    """Build the per-core Bass/Tile program (SPMD; identical on all cores)."""
    NT = bc // 512
    BT = bc // 128
    DR = mybir.MatmulPerfMode.DoubleRow

    nc = bacc.Bacc(None, target_bir_lowering=False, debug=False)
    dbg = {}
    if dump_acts:
        for nm in ("act1d", "act2d", "act3d", "act4d"):
            cdim = 3 if nm == "act1d" else 16
            dbg[nm] = nc.dram_tensor(
                nm, [128, cdim, 2, bc], FP8, kind="ExternalOutput")
        dbg["h4d"] = nc.dram_tensor("h4d", [16, bc], FP32, kind="ExternalOutput")

    xt = nc.dram_tensor("xt", [IND, bc], BF16, kind="ExternalInput")
    w1 = nc.dram_tensor("w1dr", [32, 128, 3, 2, 128], FP8, kind="ExternalInput")
    w2 = nc.dram_tensor("w2dr", [32, 128, 16, 2, 128], FP8, kind="ExternalInput")
    w3 = nc.dram_tensor("w3dr", [32, 128, 16, 2, 128], FP8, kind="ExternalInput")
    w4 = nc.dram_tensor("w4dr", [128, 16, 2, 16], FP8, kind="ExternalInput")
    thr1 = nc.dram_tensor("thr1", [128, 32], FP32, kind="ExternalInput")
    thr2 = nc.dram_tensor("thr2", [128, 32], FP32, kind="ExternalInput")
    thr3 = nc.dram_tensor("thr3", [128, 32], FP32, kind="ExternalInput")
    c4 = nc.dram_tensor("c4", [16, 2], FP32, kind="ExternalInput")
    out = nc.dram_tensor("out", [bc, OUT], FP32, kind="ExternalOutput")

    with tile.TileContext(nc) as tc, ExitStack() as ctx:
        consts = ctx.enter_context(tc.tile_pool(name="consts", bufs=1))
        xpool = ctx.enter_context(tc.tile_pool(name="xpool", bufs=2))
        a1pool = ctx.enter_context(tc.tile_pool(name="a1pool", bufs=1))
        apool = ctx.enter_context(
            tc.tile_pool(name="apool", bufs=3 if dump_acts else 2))
        wpool = ctx.enter_context(tc.tile_pool(name="wpool", bufs=3))
        smpool = ctx.enter_context(tc.tile_pool(name="smpool", bufs=3))
        psum_pool = ctx.enter_context(
            tc.tile_pool(name="psum", bufs=8, space="PSUM"))

        thr1_sb = consts.tile([128, 32], FP32, tag="thr1")
        thr2_sb = consts.tile([128, 32], FP32, tag="thr2")
        thr3_sb = consts.tile([128, 32], FP32, tag="thr3")
        c4_sb = consts.tile([16, 2], FP32, tag="c4")
        w4_sb = consts.tile([128, 16, 2, 16], FP8, tag="w4")
        ident = consts.tile([128, 128], FP32, tag="ident")
        h4 = consts.tile([16, bc], FP32, tag="h4")
        out_sb = consts.tile([128, BT, OUT], FP32, tag="outsb")

        # consts go on the scalar HWDGE ring so x owns the sync ring from t=0
        nc.scalar.dma_start(out=thr1_sb[:], in_=thr1[:])
        nc.scalar.dma_start(out=thr2_sb[:], in_=thr2[:])
        nc.scalar.dma_start(out=thr3_sb[:], in_=thr3[:])
        nc.scalar.dma_start(out=c4_sb[:], in_=c4[:])
        nc.scalar.dma_start(out=w4_sb[:], in_=w4[:])
        make_identity(nc, ident[:])

        # ---- binarize x: act1 = [x >= 0] in {0,1} fp8, feature-major.
        # x tiles alternate between the two HWDGE rings to halve the
        # serial DMA latency on the critical path into layer 1.
        act1 = a1pool.tile([128, 3, 2, bc], FP8, tag="act1")
        for i in range(6):
            xt_t = xpool.tile([128, bc], BF16, tag="x")
            eng = nc.sync if i % 2 == 0 else nc.scalar
            eng.dma_start(out=xt_t[:], in_=xt[128 * i:128 * (i + 1), :])
            nc.vector.tensor_scalar(
                out=act1[:, i // 2, i % 2, :],
                in0=xt_t[:],
                scalar1=0.0,
                scalar2=None,
                op0=mybir.AluOpType.is_ge,
            )

        # ---- layers 1-3 ----
        act2 = apool.tile([128, 16, 2, bc], FP8, tag="actbig")
        _layer_fwd(nc, wpool, psum_pool, act1, 3, w1, thr1_sb, act2, 32, bc,
                   dma_engine=nc.scalar)
        act3 = apool.tile([128, 16, 2, bc], FP8, tag="actbig")
        _layer_fwd(nc, wpool, psum_pool, act2, 16, w2, thr2_sb, act3, 32, bc)
        act4 = apool.tile([128, 16, 2, bc], FP8, tag="actbig")
        _layer_fwd(nc, wpool, psum_pool, act3, 16, w3, thr3_sb, act4, 32, bc)

        # ---- layer 4: logits (M padded 10->16), affine folds BN+rowsum.
        # Softmax is phased to avoid ACT table thrash (Exp/Ln swaps).
        sh = smpool.tile([128, BT, OUT], FP32, tag="sh", bufs=1)
        se = smpool.tile([128, BT], FP32, tag="se", bufs=1)
        ls = smpool.tile([128, BT], FP32, tag="ls", bufs=1)
        for n in range(NT):
            ps4 = psum_pool.tile([16, 512], FP32, tag="psum", name=f"ps4_{n}")
            for c in range(16):
                nc.tensor.matmul(
                    ps4[:],
                    lhsT=w4_sb[:, c, :, :],
                    rhs=act4[:, c, :, 512 * n:512 * (n + 1)],
                    start=(c == 0),
                    stop=(c == 15),
                    perf_mode=DR,
                )
            nc.vector.tensor_scalar(
                out=h4[:, 512 * n:512 * (n + 1)],
                in0=ps4[:],
                scalar1=c4_sb[:, 0:1],
                scalar2=c4_sb[:, 1:2],
                op0=mybir.AluOpType.mult,
                op1=mybir.AluOpType.add,
            )
        # transposes emitted after all L4 matmuls so the PE never waits on
        # the DVE affine between n-groups
        for bt in range(BT):
            tp = psum_pool.tile([128, OUT], FP32, tag="psum",
                                name=f"tp{bt}")
            nc.tensor.transpose(
                tp[:], h4[0:OUT, 128 * bt:128 * (bt + 1)],
                ident[0:OUT, 0:OUT])
            mx = smpool.tile([128, 1], FP32, tag="mx")
            nc.vector.reduce_max(mx[:], tp[:], axis=mybir.AxisListType.X)
            nc.vector.tensor_scalar(
                out=sh[:, bt, :], in0=tp[:], scalar1=mx[:], scalar2=None,
                op0=mybir.AluOpType.subtract)
        ex = smpool.tile([128, BT, OUT], FP32, tag="ex", bufs=1)
        for bt in range(BT):  # all Exp together: one ACT table load
            nc.scalar.activation(
                ex[:, bt, :], sh[:, bt, :], mybir.ActivationFunctionType.Exp,
                accum_out=se[:, bt:bt + 1])
        nc.scalar.activation(  # single Ln over all batch tiles
            ls[:], se[:], mybir.ActivationFunctionType.Ln)
        for bt in range(BT):
            nc.vector.tensor_scalar(
                out=out_sb[:, bt, :], in0=sh[:, bt, :],
                scalar1=ls[:, bt:bt + 1], scalar2=None,
                op0=mybir.AluOpType.subtract)

        nc.sync.dma_start(
            out=out.rearrange("(bt p) o -> p bt o", p=128), in_=out_sb[:])

        if dump_acts:
            nc.sync.dma_start(out=dbg["act1d"][:], in_=act1[:])
            nc.sync.dma_start(out=dbg["act2d"][:], in_=act2[:])
            nc.sync.dma_start(out=dbg["act3d"][:], in_=act3[:])
            nc.sync.dma_start(out=dbg["act4d"][:], in_=act4[:])
            nc.sync.dma_start(out=dbg["h4d"][:], in_=h4[:])

    nc.compile()
    return nc


# --------------------------------------------------------------------------
# Host-side preparation
# --------------------------------------------------------------------------

def _pack_w_dr(ws_t):
    """[Fin, Fout] {-1,+1} -> [Mt, 128, C, 2, 128] fp8 DoubleRow layout.

    wdr[mt, ki, c, ko, mi] = ws_t[256*c + 128*ko + ki, 128*mt + mi]
    """
    fin, fout = ws_t.shape
    C, Mt = fin // 256, fout // 128
    w = ws_t.reshape(C, 2, 128, Mt, 128).transpose(3, 2, 0, 1, 4)
    return np.ascontiguousarray(w).astype(NP_FP8)


def prepare_consts(inputs):
    """Fold sign(w), BN, bias and the 0/1-activation rowsum correction.

    The device computes, per layer, a_dev = [mmA~ >= thr] where
    mmA~ = W~sign @ a_dev_prev over {0,1} activations. Negative BN scales
    (alpha <= 0) are handled exactly by tracking a per-neuron flip bit
    (a_true = 1 - a_dev) that folds into the *next* layer's weight signs:
    with s~ = s * (1-2*flip_in), mm_full = 2*(s~ @ a_dev) - rowsum(s~)
    holds for any flip pattern. Thresholds use integer snapping (mmA is
    always an integer), making the device comparison tie-free/exact.
    """
    consts = {}
    flip_in = np.zeros(IND)  # input layer: a_dev = [x >= 0] = ste_sign, exact
    for i in (1, 2, 3, 4):
        w = np.asarray(inputs[f"w{i}"]).astype(np.float64)
        b = np.asarray(inputs[f"b{i}"]).astype(np.float64)
        g = np.asarray(inputs[f"g{i}"]).astype(np.float64)
        be = np.asarray(inputs[f"be{i}"]).astype(np.float64)
        m = np.asarray(inputs[f"m{i}"]).astype(np.float64)
        v = np.asarray(inputs[f"v{i}"]).astype(np.float64)
        ws = np.where(w >= 0, 1.0, -1.0) * (1.0 - 2.0 * flip_in)  # [fo, fi]
        rowsum = ws.sum(axis=1)                                   # [fo]
        alpha = g / np.sqrt(v + EPS)
        if i < 4:
            # BN(mm_full + b) >= 0 with mm_full = 2*mmA - rowsum:
            #   alpha > 0:  a_true = [mmA >= u],  u = (m-b-be/a+rowsum)/2
            #   alpha < 0:  a_true = [mmA <= u] = 1 - [mmA >= floor(u)+1]
            #   alpha == 0: BN = be, constant sign
            u = (m - b - be / alpha_safe(alpha) + rowsum) / 2.0
            pos = alpha > 0
            thr = np.where(pos, np.ceil(u), np.floor(u) + 1.0)
            zero = alpha == 0
            if zero.any():
                # constant: a_true = [be >= 0]; force a_dev accordingly
                thr = np.where(zero & (be >= 0), -1e30, thr)
                thr = np.where(zero & (be < 0), 1e30, thr)
                pos = pos | zero
            flip_in = (~pos).astype(np.float64)
            consts[f"thr{i}"] = np.ascontiguousarray(
                thr.reshape(32, 128).T).astype(np.float32)
            consts[f"w{i}dr"] = _pack_w_dr(ws.T)
        else:
            # logits = mmA*(2*alpha) + ((b - m - rowsum)*alpha + be), pad to 16
            scale = 2.0 * alpha
            beta = (b - m - rowsum) * alpha + be
            c4 = np.zeros((16, 2), np.float32)
            c4[:10, 0] = scale.astype(np.float32)
            c4[:10, 1] = beta.astype(np.float32)
            consts["c4"] = c4
            ws_t_pad = np.zeros((HID, 16), np.float64)
            ws_t_pad[:, :10] = ws.T
            # w4dr[ki, c, ko, m] = ws_t_pad[256*c + 128*ko + ki, m]
            w4 = ws_t_pad.reshape(16, 2, 128, 16).transpose(2, 0, 1, 3)
            consts["w4dr"] = np.ascontiguousarray(w4).astype(NP_FP8)
    return consts


def alpha_safe(a):
    return np.where(a == 0, 1.0, a)


_PROG_CACHE = {}


def _get_program(bc=BC):
    if bc not in _PROG_CACHE:
        _PROG_CACHE[bc] = build_program(bc)
    return _PROG_CACHE[bc]


def kernel(**inputs):
    global LAST_RESULTS
    x = np.asarray(inputs["x"], np.float32)
    assert x.shape == (B, 784)
    consts = prepare_consts(inputs)
    # bf16 halves the x DMA; sign(x) is unaffected (bf16 RNE preserves sign
    # for all float32 normals, and -0.0 >= 0 is true in both).
    xt_full = np.ascontiguousarray(x[:, :IND].T).astype(NP_BF16)  # [768, B]

    nc = _get_program(BC)
    in_maps = []
    for c in range(N_CORES):
        m = {"xt": np.ascontiguousarray(xt_full[:, c * BC:(c + 1) * BC])}
        m.update(consts)
        in_maps.append(m)

    res = run_bass_kernel_spmd(
        nc, in_maps, core_ids=list(range(N_CORES)), trace=TRACE,
        **TRACE_KWARGS)
    LAST_RESULTS = res
    return np.concatenate([r["out"] for r in res.results], axis=0)


# revision 31
# speedup vs baseline: 1.0481x; 1.0055x over previous
"""Binarized-MLP (BinaryNet) forward on 8 Trainium2 NeuronCores.

Reference computation (per nn_FC_large):
    h = sign(x[:, :768]) @ sign(w1).T + b1 ; BN1 ; -> sign
    h = sign(h) @ sign(w2).T + b2         ; BN2 ; -> sign
    h = sign(h) @ sign(w3).T + b3         ; BN3 ; -> sign
    h = sign(h) @ sign(w4).T + b4         ; BN4 ; log_softmax

Strategy (data parallel, batch 16384 -> 2048 rows/core):
  * All matmul operands are exactly representable in fp8: weights are
    binarized host-side to {-1,+1}; activations are kept as a in {0,1}
    (a = [pre-act >= 0]) and the identity
        sign_mm = 2*(Wsign @ a) - rowsum(Wsign)
    folds rowsum into per-neuron thresholds, so each layer's epilogue is a
    single DVE is_ge producing the next layer's {0,1} fp8 activations.
  * Matmuls run in fp8e4 with perf_mode=DoubleRow (K=256 per instruction),
    activations stored feature-major [F, B] in SBUF across the whole net.
  * BatchNorm (eval) + bias fold into thresholds (layers 1-3) / an affine
    (layer 4). Layer-4 logits are PE-transposed to batch-major and
    log_softmax runs on-device (DVE/ACT).
  * Accumulation is exact: products are in {-1,0,1}, sums are integers
    well inside fp32, so the binary pipeline is bit-exact w.r.t. the
    reference up to thresholds ties (probability ~0 with random BN stats).

Everything is hardcoded for x:[16384,784], layers 768->4096->4096->4096->10.
"""

import numpy as np
import ml_dtypes
from contextlib import ExitStack

import concourse.mybir as mybir
import concourse.tile as tile
from concourse import bacc
from concourse.bass_utils import run_bass_kernel_spmd
from concourse.masks import make_identity

FP32 = mybir.dt.float32
BF16 = mybir.dt.bfloat16
FP8 = mybir.dt.float8e4
NP_FP8 = ml_dtypes.float8_e4m3
NP_BF16 = ml_dtypes.bfloat16

EPS = 1e-5
B, IND, HID, OUT = 16384, 768, 4096, 10
N_CORES = 8
BC = B // N_CORES  # 2048 batch rows per core

# Knobs (test.py may flip TRACE before calling kernel()).
TRACE = False
TRACE_KWARGS = {}
LAST_RESULTS = None  # BassKernelResults of the most recent run


# --------------------------------------------------------------------------
# Device program
# --------------------------------------------------------------------------

def _layer_fwd(nc, wpool, psum_pool, act_in, C, wdr, thr_sb, act_out, Mt, bc,
               dma_engine=None):
    """One binarized layer: act_out = [W_fp8dr.T @ act_in >= thr] in {0,1} fp8.

    act_in : SBUF AP [128, C, 2, bc] fp8 ({0,1})
    wdr    : DRAM [Mt, 128, C, 2, 128] fp8 ({-1,+1})
    thr_sb : SBUF [128, Mt] fp32
    act_out: SBUF AP [128, Mt//2, 2, bc] fp8
    """
    NT = bc // 512
    DR = mybir.MatmulPerfMode.DoubleRow
    dma_engine = dma_engine or nc.sync
    for mt in range(Mt):
        wt = wpool.tile([128, C, 2, 128], FP8, tag="w")
        dma_engine.dma_start(out=wt[:], in_=wdr[mt])
        pss = [psum_pool.tile([128, 512], FP32, tag="psum", name=f"ps{mt}_{n}")
               for n in range(NT)]
        for c in range(C):
            for n in range(NT):
                nc.tensor.matmul(
                    pss[n][:],
                    lhsT=wt[:, c, :, :],
                    rhs=act_in[:, c, :, 512 * n:512 * (n + 1)],
                    start=(c == 0),
                    stop=(c == C - 1),
                    perf_mode=DR,
                )
        for n in range(NT):
            nc.vector.tensor_scalar(
                out=act_out[:, mt // 2, mt % 2, 512 * n:512 * (n + 1)],
                in0=pss[n][:],
                scalar1=thr_sb[:, mt:mt + 1],
                scalar2=None,
                op0=mybir.AluOpType.is_ge,
            )


def build_program(bc=BC, dump_acts=False):
    """Build the per-core Bass/Tile program (SPMD; identical on all cores)."""
    NT = bc // 512
    BT = bc // 128
    DR = mybir.MatmulPerfMode.DoubleRow

    nc = bacc.Bacc(None, target_bir_lowering=False, debug=False)
    dbg = {}
    if dump_acts:
        for nm in ("act1d", "act2d", "act3d", "act4d"):
            cdim = 3 if nm == "act1d" else 16
            dbg[nm] = nc.dram_tensor(
                nm, [128, cdim, 2, bc], FP8, kind="ExternalOutput")
        dbg["h4d"] = nc.dram_tensor("h4d", [16, bc], FP32, kind="ExternalOutput")

    xt = nc.dram_tensor("xt", [IND, bc], BF16, kind="ExternalInput")
    w1 = nc.dram_tensor("w1dr", [32, 128, 3, 2, 128], FP8, kind="ExternalInput")
    w2 = nc.dram_tensor("w2dr", [32, 128, 16, 2, 128], FP8, kind="ExternalInput")
    w3 = nc.dram_tensor("w3dr", [32, 128, 16, 2, 128], FP8, kind="ExternalInput")
    w4 = nc.dram_tensor("w4dr", [128, 16, 2, 16], FP8, kind="ExternalInput")
    thrs = nc.dram_tensor("thrs", [128, 3, 32], FP32, kind="ExternalInput")
    c4 = nc.dram_tensor("c4", [16, 2], FP32, kind="ExternalInput")
    out = nc.dram_tensor("out", [128, bc // 128, OUT], FP32,
                         kind="ExternalOutput")

    with tile.TileContext(nc) as tc, ExitStack() as ctx:
        consts = ctx.enter_context(tc.tile_pool(name="consts", bufs=1))
        xpool = ctx.enter_context(tc.tile_pool(name="xpool", bufs=2))
        a1pool = ctx.enter_context(tc.tile_pool(name="a1pool", bufs=1))
        apool = ctx.enter_context(
            tc.tile_pool(name="apool", bufs=3 if dump_acts else 2))
        wpool = ctx.enter_context(tc.tile_pool(name="wpool", bufs=3))
        smpool = ctx.enter_context(tc.tile_pool(name="smpool", bufs=3))
        psum_pool = ctx.enter_context(
            tc.tile_pool(name="psum", bufs=8, space="PSUM"))

        thrs_sb = consts.tile([128, 3, 32], FP32, tag="thrs")
        c4_sb = consts.tile([16, 2], FP32, tag="c4")
        w4_sb = consts.tile([128, 16, 2, 16], FP8, tag="w4")
        ident = consts.tile([128, 128], FP32, tag="ident")
        h4 = consts.tile([16, bc], FP32, tag="h4")
        out_sb = consts.tile([128, BT, OUT], FP32, tag="outsb")
        thr1_sb = thrs_sb[:, 0, :]
        thr2_sb = thrs_sb[:, 1, :]
        thr3_sb = thrs_sb[:, 2, :]

        # ---- binarize x: act1 = [x >= 0] in {0,1} fp8, feature-major.
        # HWDGE descriptor-gen is ~0.65us *serial* per dma_start, so x goes
        # as TWO batched strided DMAs (one per ring), emitted before all
        # const DMAs. xa carries k-chunk c0 (the layer-1 critical path),
        # xb carries c1+c2; binarize is one DVE op per k-chunk.
        act1 = a1pool.tile([128, 3, 2, bc], FP8, tag="act1")
        xa = xpool.tile([128, 2, bc], BF16, tag="xa", bufs=1)
        xb = xpool.tile([128, 4, bc], BF16, tag="xb", bufs=1)
        nc.sync.dma_start(
            out=xa[:], in_=xt[0:256, :].rearrange("(i p) n -> p i n", p=128))
        nc.scalar.dma_start(
            out=xb[:], in_=xt[256:768, :].rearrange("(i p) n -> p i n", p=128))

        # consts follow x on the rings; their data is needed much later
        nc.scalar.dma_start(out=thrs_sb[:], in_=thrs[:])
        nc.scalar.dma_start(out=c4_sb[:], in_=c4[:])
        nc.scalar.dma_start(out=w4_sb[:], in_=w4[:])
        make_identity(nc, ident[:])

        nc.vector.tensor_scalar(
            out=act1[:, 0, :, :], in0=xa[:], scalar1=0.0, scalar2=None,
            op0=mybir.AluOpType.is_ge)
        for c in (1, 2):
            nc.vector.tensor_scalar(
                out=act1[:, c, :, :], in0=xb[:, 2 * (c - 1):2 * c, :],
                scalar1=0.0, scalar2=None, op0=mybir.AluOpType.is_ge)

        # ---- layers 1-3 ----
        act2 = apool.tile([128, 16, 2, bc], FP8, tag="actbig")
        _layer_fwd(nc, wpool, psum_pool, act1, 3, w1, thr1_sb, act2, 32, bc,
                   dma_engine=nc.scalar)
        act3 = apool.tile([128, 16, 2, bc], FP8, tag="actbig")
        _layer_fwd(nc, wpool, psum_pool, act2, 16, w2, thr2_sb, act3, 32, bc)
        act4 = apool.tile([128, 16, 2, bc], FP8, tag="actbig")
        _layer_fwd(nc, wpool, psum_pool, act3, 16, w3, thr3_sb, act4, 32, bc)

        # ---- layer 4: logits (M padded 10->16), affine folds BN+rowsum.
        # Softmax is phased to avoid ACT table thrash (Exp/Ln swaps).
        sh = smpool.tile([128, BT, OUT], FP32, tag="sh", bufs=1)
        se = smpool.tile([128, BT], FP32, tag="se", bufs=1)
        ls = smpool.tile([128, BT], FP32, tag="ls", bufs=1)

        def _l4_softmax_head(g):
            # transpose group g's batch tiles + max/shift on DVE; runs one
            # n-group behind the L4 matmuls so the PE never stalls on it
            for bt in range(4 * g, 4 * g + 4):
                tp = psum_pool.tile([128, OUT], FP32, tag="psum",
                                    name=f"tp{bt}")
                nc.tensor.transpose(
                    tp[:], h4[0:OUT, 128 * bt:128 * (bt + 1)],
                    ident[0:OUT, 0:OUT])
                mx = smpool.tile([128, 1], FP32, tag="mx", name=f"mx{bt}")
                nc.vector.reduce_max(mx[:], tp[:], axis=mybir.AxisListType.X)
                nc.vector.tensor_scalar(
                    out=sh[:, bt, :], in0=tp[:], scalar1=mx[:], scalar2=None,
                    op0=mybir.AluOpType.subtract)
        for n in range(NT):
            ps4 = psum_pool.tile([16, 512], FP32, tag="psum", name=f"ps4_{n}")
            for c in range(16):
                nc.tensor.matmul(
                    ps4[:],
                    lhsT=w4_sb[:, c, :, :],
                    rhs=act4[:, c, :, 512 * n:512 * (n + 1)],
                    start=(c == 0),
                    stop=(c == 15),
                    perf_mode=DR,
                )
            nc.vector.tensor_scalar(
                out=h4[:, 512 * n:512 * (n + 1)],
                in0=ps4[:],
                scalar1=c4_sb[:, 0:1],
                scalar2=c4_sb[:, 1:2],
                op0=mybir.AluOpType.mult,
                op1=mybir.AluOpType.add,
            )
        for g in range(NT):
            _l4_softmax_head(g)
        ex = smpool.tile([128, BT, OUT], FP32, tag="ex", bufs=1)
        for bt in range(BT):  # all Exp together: one ACT table load
            nc.scalar.activation(
                ex[:, bt, :], sh[:, bt, :], mybir.ActivationFunctionType.Exp,
                accum_out=se[:, bt:bt + 1])
        nc.scalar.activation(  # single Ln over all batch tiles
            ls[:], se[:], mybir.ActivationFunctionType.Ln)
        for bt in range(BT):
            nc.vector.tensor_scalar(
                out=out_sb[:, bt, :], in0=sh[:, bt, :],
                scalar1=ls[:, bt:bt + 1], scalar2=None,
                op0=mybir.AluOpType.subtract)

        # out dram is [128, BT, OUT] (partition-major, fully contiguous DMA);
        # the host reassembles batch order with a free transpose.
        nc.sync.dma_start(out=out[:], in_=out_sb[:])

        if dump_acts:
            nc.sync.dma_start(out=dbg["act1d"][:], in_=act1[:])
            nc.sync.dma_start(out=dbg["act2d"][:], in_=act2[:])
            nc.sync.dma_start(out=dbg["act3d"][:], in_=act3[:])
            nc.sync.dma_start(out=dbg["act4d"][:], in_=act4[:])
            nc.sync.dma_start(out=dbg["h4d"][:], in_=h4[:])

    nc.compile()
    return nc


# --------------------------------------------------------------------------
# Host-side preparation
# --------------------------------------------------------------------------

def _pack_w_dr(ws_t):
    """[Fin, Fout] {-1,+1} -> [Mt, 128, C, 2, 128] fp8 DoubleRow layout.

    wdr[mt, ki, c, ko, mi] = ws_t[256*c + 128*ko + ki, 128*mt + mi]
    """
    fin, fout = ws_t.shape
    C, Mt = fin // 256, fout // 128
    w = ws_t.reshape(C, 2, 128, Mt, 128).transpose(3, 2, 0, 1, 4)
    return np.ascontiguousarray(w).astype(NP_FP8)


def prepare_consts(inputs):
    """Fold sign(w), BN, bias and the 0/1-activation rowsum correction.

    The device computes, per layer, a_dev = [mmA~ >= thr] where
    mmA~ = W~sign @ a_dev_prev over {0,1} activations. Negative BN scales
    (alpha <= 0) are handled exactly by tracking a per-neuron flip bit
    (a_true = 1 - a_dev) that folds into the *next* layer's weight signs:
    with s~ = s * (1-2*flip_in), mm_full = 2*(s~ @ a_dev) - rowsum(s~)
    holds for any flip pattern. Thresholds use integer snapping (mmA is
    always an integer), making the device comparison tie-free/exact.
    """
    consts = {}
    flip_in = np.zeros(IND)  # input layer: a_dev = [x >= 0] = ste_sign, exact
    for i in (1, 2, 3, 4):
        w = np.asarray(inputs[f"w{i}"]).astype(np.float64)
        b = np.asarray(inputs[f"b{i}"]).astype(np.float64)
        g = np.asarray(inputs[f"g{i}"]).astype(np.float64)
        be = np.asarray(inputs[f"be{i}"]).astype(np.float64)
        m = np.asarray(inputs[f"m{i}"]).astype(np.float64)
        v = np.asarray(inputs[f"v{i}"]).astype(np.float64)
        ws = np.where(w >= 0, 1.0, -1.0) * (1.0 - 2.0 * flip_in)  # [fo, fi]
        rowsum = ws.sum(axis=1)                                   # [fo]
        alpha = g / np.sqrt(v + EPS)
        if i < 4:
            # BN(mm_full + b) >= 0 with mm_full = 2*mmA - rowsum:
            #   alpha > 0:  a_true = [mmA >= u],  u = (m-b-be/a+rowsum)/2
            #   alpha < 0:  a_true = [mmA <= u] = 1 - [mmA >= floor(u)+1]
            #   alpha == 0: BN = be, constant sign
            u = (m - b - be / alpha_safe(alpha) + rowsum) / 2.0
            pos = alpha > 0
            thr = np.where(pos, np.ceil(u), np.floor(u) + 1.0)
            zero = alpha == 0
            if zero.any():
                # constant: a_true = [be >= 0]; force a_dev accordingly
                thr = np.where(zero & (be >= 0), -1e30, thr)
                thr = np.where(zero & (be < 0), 1e30, thr)
                pos = pos | zero
            flip_in = (~pos).astype(np.float64)
            consts.setdefault("_thrs", []).append(
                thr.reshape(32, 128).T.astype(np.float32))
            consts[f"w{i}dr"] = _pack_w_dr(ws.T)
        else:
            # logits = mmA*(2*alpha) + ((b - m - rowsum)*alpha + be), pad to 16
            scale = 2.0 * alpha
            beta = (b - m - rowsum) * alpha + be
            c4 = np.zeros((16, 2), np.float32)
            c4[:10, 0] = scale.astype(np.float32)
            c4[:10, 1] = beta.astype(np.float32)
            consts["c4"] = c4
            ws_t_pad = np.zeros((HID, 16), np.float64)
            ws_t_pad[:, :10] = ws.T
            # w4dr[ki, c, ko, m] = ws_t_pad[256*c + 128*ko + ki, m]
            w4 = ws_t_pad.reshape(16, 2, 128, 16).transpose(2, 0, 1, 3)
            consts["w4dr"] = np.ascontiguousarray(w4).astype(NP_FP8)
    consts["thrs"] = np.ascontiguousarray(
        np.stack(consts.pop("_thrs"), axis=1))  # [128, 3, 32]
    return consts


def alpha_safe(a):
    return np.where(a == 0, 1.0, a)


_PROG_CACHE = {}


def _get_program(bc=BC):
    if bc not in _PROG_CACHE:
        _PROG_CACHE[bc] = build_program(bc)
    return _PROG_CACHE[bc]


def kernel(**inputs):
    global LAST_RESULTS
    x = np.asarray(inputs["x"], np.float32)
    assert x.shape == (B, 784)
    consts = prepare_consts(inputs)
    # bf16 halves the x DMA; sign(x) is unaffected (bf16 RNE preserves sign
    # for all float32 normals, and -0.0 >= 0 is true in both).
    xt_full = np.ascontiguousarray(x[:, :IND].T).astype(NP_BF16)  # [768, B]

    nc = _get_program(BC)
    in_maps = []
    for c in range(N_CORES):
        m = {"xt": np.ascontiguousarray(xt_full[:, c * BC:(c + 1) * BC])}
        m.update(consts)
        in_maps.append(m)

    res = run_bass_kernel_spmd(
        nc, in_maps, core_ids=list(range(N_CORES)), trace=TRACE,
        **TRACE_KWARGS)
    LAST_RESULTS = res
    # device out is [128, BT, 10] partition-major; restore batch order
    outs = [np.ascontiguousarray(r["out"].transpose(1, 0, 2).reshape(BC, OUT))
            for r in res.results]
    return np.concatenate(outs, axis=0)


# revision 32
# speedup vs baseline: 1.0499x; 1.0016x over previous
"""Binarized-MLP (BinaryNet) forward on 8 Trainium2 NeuronCores.

Reference computation (per nn_FC_large):
    h = sign(x[:, :768]) @ sign(w1).T + b1 ; BN1 ; -> sign
    h = sign(h) @ sign(w2).T + b2         ; BN2 ; -> sign
    h = sign(h) @ sign(w3).T + b3         ; BN3 ; -> sign
    h = sign(h) @ sign(w4).T + b4         ; BN4 ; log_softmax

Strategy (data parallel, batch 16384 -> 2048 rows/core):
  * All matmul operands are exactly representable in fp8: weights are
    binarized host-side to {-1,+1}; activations are kept as a in {0,1}
    (a = [pre-act >= 0]) and the identity
        sign_mm = 2*(Wsign @ a) - rowsum(Wsign)
    folds rowsum into per-neuron thresholds, so each layer's epilogue is a
    single DVE is_ge producing the next layer's {0,1} fp8 activations.
  * Matmuls run in fp8e4 with perf_mode=DoubleRow (K=256 per instruction),
    activations stored feature-major [F, B] in SBUF across the whole net.
  * BatchNorm (eval) + bias fold into thresholds (layers 1-3) / an affine
    (layer 4). Layer-4 logits are PE-transposed to batch-major and
    log_softmax runs on-device (DVE/ACT).
  * Accumulation is exact: products are in {-1,0,1}, sums are integers
    well inside fp32, so the binary pipeline is bit-exact w.r.t. the
    reference up to thresholds ties (probability ~0 with random BN stats).

Everything is hardcoded for x:[16384,784], layers 768->4096->4096->4096->10.
"""

import numpy as np
import ml_dtypes
from contextlib import ExitStack

import concourse.mybir as mybir
import concourse.tile as tile
from concourse import bacc
from concourse.bass_utils import run_bass_kernel_spmd
from concourse.masks import make_identity

FP32 = mybir.dt.float32
BF16 = mybir.dt.bfloat16
FP8 = mybir.dt.float8e4
NP_FP8 = ml_dtypes.float8_e4m3
NP_BF16 = ml_dtypes.bfloat16

EPS = 1e-5
B, IND, HID, OUT = 16384, 768, 4096, 10
N_CORES = 8
BC = B // N_CORES  # 2048 batch rows per core

# Knobs (test.py may flip TRACE before calling kernel()).
TRACE = False
TRACE_KWARGS = {}
LAST_RESULTS = None  # BassKernelResults of the most recent run


# --------------------------------------------------------------------------
# Device program
# --------------------------------------------------------------------------

def _layer_fwd(nc, wpool, psum_pool, act_in, C, wdr, thr_sb, act_out, Mt, bc,
               dma_engine=None):
    """One binarized layer: act_out = [W_fp8dr.T @ act_in >= thr] in {0,1} fp8.

    act_in : SBUF AP [128, C, 2, bc] fp8 ({0,1})
    wdr    : DRAM [Mt, 128, C, 2, 128] fp8 ({-1,+1})
    thr_sb : SBUF [128, Mt] fp32
    act_out: SBUF AP [128, Mt//2, 2, bc] fp8
    """
    NT = bc // 512
    DR = mybir.MatmulPerfMode.DoubleRow
    dma_engine = dma_engine or nc.sync
    for mt in range(Mt):
        wt = wpool.tile([128, C, 2, 128], FP8, tag="w")
        dma_engine.dma_start(out=wt[:], in_=wdr[mt])
        pss = [psum_pool.tile([128, 512], FP32, tag="psum", name=f"ps{mt}_{n}")
               for n in range(NT)]
        for c in range(C):
            for n in range(NT):
                nc.tensor.matmul(
                    pss[n][:],
                    lhsT=wt[:, c, :, :],
                    rhs=act_in[:, c, :, 512 * n:512 * (n + 1)],
                    start=(c == 0),
                    stop=(c == C - 1),
                    perf_mode=DR,
                )
        for n in range(NT):
            nc.vector.tensor_scalar(
                out=act_out[:, mt // 2, mt % 2, 512 * n:512 * (n + 1)],
                in0=pss[n][:],
                scalar1=thr_sb[:, mt:mt + 1],
                scalar2=None,
                op0=mybir.AluOpType.is_ge,
            )


def build_program(bc=BC, dump_acts=False):
    """Build the per-core Bass/Tile program (SPMD; identical on all cores)."""
    NT = bc // 512
    BT = bc // 128
    DR = mybir.MatmulPerfMode.DoubleRow

    nc = bacc.Bacc(None, target_bir_lowering=False, debug=False)
    dbg = {}
    if dump_acts:
        for nm in ("act1d", "act2d", "act3d", "act4d"):
            cdim = 3 if nm == "act1d" else 16
            dbg[nm] = nc.dram_tensor(
                nm, [128, cdim, 2, bc], FP8, kind="ExternalOutput")
        dbg["h4d"] = nc.dram_tensor("h4d", [16, bc], FP32, kind="ExternalOutput")

    xt = nc.dram_tensor("xt", [IND, bc], BF16, kind="ExternalInput")
    w1 = nc.dram_tensor("w1dr", [32, 128, 3, 2, 128], FP8, kind="ExternalInput")
    w2 = nc.dram_tensor("w2dr", [32, 128, 16, 2, 128], FP8, kind="ExternalInput")
    w3 = nc.dram_tensor("w3dr", [32, 128, 16, 2, 128], FP8, kind="ExternalInput")
    w4 = nc.dram_tensor("w4dr", [128, 16, 2, 16], FP8, kind="ExternalInput")
    thrs = nc.dram_tensor("thrs", [128, 3, 32], FP32, kind="ExternalInput")
    c4 = nc.dram_tensor("c4", [16, 2], FP32, kind="ExternalInput")
    out = nc.dram_tensor("out", [128, bc // 128, OUT], FP32,
                         kind="ExternalOutput")

    with tile.TileContext(nc) as tc, ExitStack() as ctx:
        consts = ctx.enter_context(tc.tile_pool(name="consts", bufs=1))
        xpool = ctx.enter_context(tc.tile_pool(name="xpool", bufs=2))
        a1pool = ctx.enter_context(tc.tile_pool(name="a1pool", bufs=1))
        apool = ctx.enter_context(
            tc.tile_pool(name="apool", bufs=3 if dump_acts else 2))
        wpool = ctx.enter_context(tc.tile_pool(name="wpool", bufs=4))
        smpool = ctx.enter_context(tc.tile_pool(name="smpool", bufs=3))
        psum_pool = ctx.enter_context(
            tc.tile_pool(name="psum", bufs=8, space="PSUM"))

        thrs_sb = consts.tile([128, 3, 32], FP32, tag="thrs")
        c4_sb = consts.tile([16, 2], FP32, tag="c4")
        w4_sb = consts.tile([128, 16, 2, 16], FP8, tag="w4")
        ident = consts.tile([128, 128], FP32, tag="ident")
        h4 = consts.tile([16, bc], FP32, tag="h4")
        out_sb = consts.tile([128, BT, OUT], FP32, tag="outsb")
        thr1_sb = thrs_sb[:, 0, :]
        thr2_sb = thrs_sb[:, 1, :]
        thr3_sb = thrs_sb[:, 2, :]

        # ---- binarize x: act1 = [x >= 0] in {0,1} fp8, feature-major.
        # HWDGE descriptor-gen is ~0.65us *serial* per dma_start, so x goes
        # as TWO batched strided DMAs (one per ring), emitted before all
        # const DMAs. xa carries k-chunk c0 (the layer-1 critical path),
        # xb carries c1+c2; binarize is one DVE op per k-chunk.
        act1 = a1pool.tile([128, 3, 2, bc], FP8, tag="act1")
        xa = xpool.tile([128, 2, bc], BF16, tag="xa", bufs=1)
        xb = xpool.tile([128, 4, bc], BF16, tag="xb", bufs=1)
        nc.sync.dma_start(
            out=xa[:], in_=xt[0:256, :].rearrange("(i p) n -> p i n", p=128))
        nc.scalar.dma_start(
            out=xb[:], in_=xt[256:768, :].rearrange("(i p) n -> p i n", p=128))

        # consts follow x on the rings; their data is needed much later
        nc.scalar.dma_start(out=thrs_sb[:], in_=thrs[:])
        nc.scalar.dma_start(out=c4_sb[:], in_=c4[:])
        nc.scalar.dma_start(out=w4_sb[:], in_=w4[:])
        make_identity(nc, ident[:])

        nc.vector.tensor_scalar(
            out=act1[:, 0, :, :], in0=xa[:], scalar1=0.0, scalar2=None,
            op0=mybir.AluOpType.is_ge)
        for c in (1, 2):
            nc.vector.tensor_scalar(
                out=act1[:, c, :, :], in0=xb[:, 2 * (c - 1):2 * c, :],
                scalar1=0.0, scalar2=None, op0=mybir.AluOpType.is_ge)

        # ---- layers 1-3 ----
        act2 = apool.tile([128, 16, 2, bc], FP8, tag="actbig")
        _layer_fwd(nc, wpool, psum_pool, act1, 3, w1, thr1_sb, act2, 32, bc,
                   dma_engine=nc.scalar)
        act3 = apool.tile([128, 16, 2, bc], FP8, tag="actbig")
        _layer_fwd(nc, wpool, psum_pool, act2, 16, w2, thr2_sb, act3, 32, bc)
        act4 = apool.tile([128, 16, 2, bc], FP8, tag="actbig")
        _layer_fwd(nc, wpool, psum_pool, act3, 16, w3, thr3_sb, act4, 32, bc)

        # ---- layer 4: logits (M padded 10->16), affine folds BN+rowsum.
        # Softmax is phased to avoid ACT table thrash (Exp/Ln swaps).
        sh = smpool.tile([128, BT, OUT], FP32, tag="sh", bufs=1)
        se = smpool.tile([128, BT], FP32, tag="se", bufs=1)
        ls = smpool.tile([128, BT], FP32, tag="ls", bufs=1)

        def _l4_softmax_head(g):
            # transpose group g's batch tiles + max/shift on DVE; runs one
            # n-group behind the L4 matmuls so the PE never stalls on it
            for bt in range(4 * g, 4 * g + 4):
                tp = psum_pool.tile([128, OUT], FP32, tag="psum",
                                    name=f"tp{bt}")
                nc.tensor.transpose(
                    tp[:], h4[0:OUT, 128 * bt:128 * (bt + 1)],
                    ident[0:OUT, 0:OUT])
                mx = smpool.tile([128, 1], FP32, tag="mx", name=f"mx{bt}")
                nc.vector.reduce_max(mx[:], tp[:], axis=mybir.AxisListType.X)
                nc.vector.tensor_scalar(
                    out=sh[:, bt, :], in0=tp[:], scalar1=mx[:], scalar2=None,
                    op0=mybir.AluOpType.subtract)
        for n in range(NT):
            ps4 = psum_pool.tile([16, 512], FP32, tag="psum", name=f"ps4_{n}")
            for c in range(16):
                nc.tensor.matmul(
                    ps4[:],
                    lhsT=w4_sb[:, c, :, :],
                    rhs=act4[:, c, :, 512 * n:512 * (n + 1)],
                    start=(c == 0),
                    stop=(c == 15),
                    perf_mode=DR,
                )
            # affine on the (idle) scalar engine: out = in*scale + bias
            nc.scalar.activation(
                h4[:, 512 * n:512 * (n + 1)], ps4[:],
                mybir.ActivationFunctionType.Identity,
                bias=c4_sb[:, 1:2], scale=c4_sb[:, 0:1],
            )
        for g in range(NT):
            _l4_softmax_head(g)
        ex = smpool.tile([128, BT, OUT], FP32, tag="ex", bufs=1)
        for bt in range(BT):  # all Exp together: one ACT table load
            nc.scalar.activation(
                ex[:, bt, :], sh[:, bt, :], mybir.ActivationFunctionType.Exp,
                accum_out=se[:, bt:bt + 1])
        nc.scalar.activation(  # single Ln over all batch tiles
            ls[:], se[:], mybir.ActivationFunctionType.Ln)
        for bt in range(BT):
            nc.vector.tensor_scalar(
                out=out_sb[:, bt, :], in0=sh[:, bt, :],
                scalar1=ls[:, bt:bt + 1], scalar2=None,
                op0=mybir.AluOpType.subtract)

        # out dram is [128, BT, OUT] (partition-major, fully contiguous DMA);
        # the host reassembles batch order with a free transpose.
        nc.sync.dma_start(out=out[:], in_=out_sb[:])

        if dump_acts:
            nc.sync.dma_start(out=dbg["act1d"][:], in_=act1[:])
            nc.sync.dma_start(out=dbg["act2d"][:], in_=act2[:])
            nc.sync.dma_start(out=dbg["act3d"][:], in_=act3[:])
            nc.sync.dma_start(out=dbg["act4d"][:], in_=act4[:])
            nc.sync.dma_start(out=dbg["h4d"][:], in_=h4[:])

    nc.compile()
    return nc


# --------------------------------------------------------------------------
# Host-side preparation
# --------------------------------------------------------------------------

def _pack_w_dr(ws_t):
    """[Fin, Fout] {-1,+1} -> [Mt, 128, C, 2, 128] fp8 DoubleRow layout.

    wdr[mt, ki, c, ko, mi] = ws_t[256*c + 128*ko + ki, 128*mt + mi]
    """
    fin, fout = ws_t.shape
    C, Mt = fin // 256, fout // 128
    w = ws_t.reshape(C, 2, 128, Mt, 128).transpose(3, 2, 0, 1, 4)
    return np.ascontiguousarray(w).astype(NP_FP8)


def prepare_consts(inputs):
    """Fold sign(w), BN, bias and the 0/1-activation rowsum correction.

    The device computes, per layer, a_dev = [mmA~ >= thr] where
    mmA~ = W~sign @ a_dev_prev over {0,1} activations. Negative BN scales
    (alpha <= 0) are handled exactly by tracking a per-neuron flip bit
    (a_true = 1 - a_dev) that folds into the *next* layer's weight signs:
    with s~ = s * (1-2*flip_in), mm_full = 2*(s~ @ a_dev) - rowsum(s~)
    holds for any flip pattern. Thresholds use integer snapping (mmA is
    always an integer), making the device comparison tie-free/exact.
    """
    consts = {}
    flip_in = np.zeros(IND)  # input layer: a_dev = [x >= 0] = ste_sign, exact
    for i in (1, 2, 3, 4):
        w = np.asarray(inputs[f"w{i}"]).astype(np.float64)
        b = np.asarray(inputs[f"b{i}"]).astype(np.float64)
        g = np.asarray(inputs[f"g{i}"]).astype(np.float64)
        be = np.asarray(inputs[f"be{i}"]).astype(np.float64)
        m = np.asarray(inputs[f"m{i}"]).astype(np.float64)
        v = np.asarray(inputs[f"v{i}"]).astype(np.float64)
        ws = np.where(w >= 0, 1.0, -1.0) * (1.0 - 2.0 * flip_in)  # [fo, fi]
        rowsum = ws.sum(axis=1)                                   # [fo]
        alpha = g / np.sqrt(v + EPS)
        if i < 4:
            # BN(mm_full + b) >= 0 with mm_full = 2*mmA - rowsum:
            #   alpha > 0:  a_true = [mmA >= u],  u = (m-b-be/a+rowsum)/2
            #   alpha < 0:  a_true = [mmA <= u] = 1 - [mmA >= floor(u)+1]
            #   alpha == 0: BN = be, constant sign
            u = (m - b - be / alpha_safe(alpha) + rowsum) / 2.0
            pos = alpha > 0
            thr = np.where(pos, np.ceil(u), np.floor(u) + 1.0)
            zero = alpha == 0
            if zero.any():
                # constant: a_true = [be >= 0]; force a_dev accordingly
                thr = np.where(zero & (be >= 0), -1e30, thr)
                thr = np.where(zero & (be < 0), 1e30, thr)
                pos = pos | zero
            flip_in = (~pos).astype(np.float64)
            consts.setdefault("_thrs", []).append(
                thr.reshape(32, 128).T.astype(np.float32))
            consts[f"w{i}dr"] = _pack_w_dr(ws.T)
        else:
            # logits = mmA*(2*alpha) + ((b - m - rowsum)*alpha + be), pad to 16
            scale = 2.0 * alpha
            beta = (b - m - rowsum) * alpha + be
            c4 = np.zeros((16, 2), np.float32)
            c4[:10, 0] = scale.astype(np.float32)
            c4[:10, 1] = beta.astype(np.float32)
            consts["c4"] = c4
            ws_t_pad = np.zeros((HID, 16), np.float64)
            ws_t_pad[:, :10] = ws.T
            # w4dr[ki, c, ko, m] = ws_t_pad[256*c + 128*ko + ki, m]
            w4 = ws_t_pad.reshape(16, 2, 128, 16).transpose(2, 0, 1, 3)
            consts["w4dr"] = np.ascontiguousarray(w4).astype(NP_FP8)
    consts["thrs"] = np.ascontiguousarray(
        np.stack(consts.pop("_thrs"), axis=1))  # [128, 3, 32]
    return consts


def alpha_safe(a):
    return np.where(a == 0, 1.0, a)


_PROG_CACHE = {}


def _get_program(bc=BC):
    if bc not in _PROG_CACHE:
        _PROG_CACHE[bc] = build_program(bc)
    return _PROG_CACHE[bc]


def kernel(**inputs):
    global LAST_RESULTS
    x = np.asarray(inputs["x"], np.float32)
    assert x.shape == (B, 784)
    consts = prepare_consts(inputs)
    # bf16 halves the x DMA; sign(x) is unaffected (bf16 RNE preserves sign
    # for all float32 normals, and -0.0 >= 0 is true in both).
    xt_full = np.ascontiguousarray(x[:, :IND].T).astype(NP_BF16)  # [768, B]

    nc = _get_program(BC)
    in_maps = []
    for c in range(N_CORES):
        m = {"xt": np.ascontiguousarray(xt_full[:, c * BC:(c + 1) * BC])}
        m.update(consts)
        in_maps.append(m)

    res = run_bass_kernel_spmd(
        nc, in_maps, core_ids=list(range(N_CORES)), trace=TRACE,
        **TRACE_KWARGS)
    LAST_RESULTS = res
    # device out is [128, BT, 10] partition-major; restore batch order
    outs = [np.ascontiguousarray(r["out"].transpose(1, 0, 2).reshape(BC, OUT))
            for r in res.results]
    return np.concatenate(outs, axis=0)


# revision 37
# speedup vs baseline: 1.0510x; 1.0011x over previous
"""Binarized-MLP (BinaryNet) forward on 8 Trainium2 NeuronCores.

Reference computation (per nn_FC_large):
    h = sign(x[:, :768]) @ sign(w1).T + b1 ; BN1 ; -> sign
    h = sign(h) @ sign(w2).T + b2         ; BN2 ; -> sign
    h = sign(h) @ sign(w3).T + b3         ; BN3 ; -> sign
    h = sign(h) @ sign(w4).T + b4         ; BN4 ; log_softmax

Strategy (data parallel, batch 16384 -> 2048 rows/core):
  * All matmul operands are exactly representable in fp8: weights are
    binarized host-side to {-1,+1}; activations are kept as a in {0,1}
    (a = [pre-act >= 0]) and the identity
        sign_mm = 2*(Wsign @ a) - rowsum(Wsign)
    folds rowsum into per-neuron thresholds, so each layer's epilogue is a
    single DVE is_ge producing the next layer's {0,1} fp8 activations.
  * Matmuls run in fp8e4 with perf_mode=DoubleRow (K=256 per instruction),
    activations stored feature-major [F, B] in SBUF across the whole net.
  * BatchNorm (eval) + bias fold into thresholds (layers 1-3) / an affine
    (layer 4). Layer-4 logits are PE-transposed to batch-major and
    log_softmax runs on-device (DVE/ACT).
  * Accumulation is exact: products are in {-1,0,1}, sums are integers
    well inside fp32, so the binary pipeline is bit-exact w.r.t. the
    reference up to thresholds ties (probability ~0 with random BN stats).

Everything is hardcoded for x:[16384,784], layers 768->4096->4096->4096->10.
"""

import numpy as np
import ml_dtypes
from contextlib import ExitStack

import concourse.mybir as mybir
import concourse.tile as tile
from concourse import bacc
from concourse.bass_utils import run_bass_kernel_spmd
from concourse.masks import make_identity

FP32 = mybir.dt.float32
BF16 = mybir.dt.bfloat16
FP8 = mybir.dt.float8e4
NP_FP8 = ml_dtypes.float8_e4m3
NP_BF16 = ml_dtypes.bfloat16

EPS = 1e-5
B, IND, HID, OUT = 16384, 768, 4096, 10
N_CORES = 8
BC = B // N_CORES  # 2048 batch rows per core

# Knobs (test.py may flip TRACE before calling kernel()).
TRACE = False
TRACE_KWARGS = {}
LAST_RESULTS = None  # BassKernelResults of the most recent run


# --------------------------------------------------------------------------
# Device program
# --------------------------------------------------------------------------

def _layer_fwd(nc, wpool, psum_pool, act_in, C, wdr, thr_sb, act_out, Mt, bc,
               dma_engine=None):
    """One binarized layer: act_out = [W_fp8dr.T @ act_in >= thr] in {0,1} fp8.

    act_in : SBUF AP [128, C, 2, bc] fp8 ({0,1})
    wdr    : DRAM [Mt, 128, C, 2, 128] fp8 ({-1,+1})
    thr_sb : SBUF [128, Mt] fp32
    act_out: SBUF AP [128, Mt//2, 2, bc] fp8
    """
    NT = bc // 512
    DR = mybir.MatmulPerfMode.DoubleRow
    dma_engine = dma_engine or nc.sync
    for mt in range(Mt):
        wt = wpool.tile([128, C, 2, 128], FP8, tag="w")
        dma_engine.dma_start(out=wt[:], in_=wdr[mt])
        pss = [psum_pool.tile([128, 512], FP32, tag="psum", name=f"ps{mt}_{n}")
               for n in range(NT)]
        for c in range(C):
            for n in range(NT):
                nc.tensor.matmul(
                    pss[n][:],
                    lhsT=wt[:, c, :, :],
                    rhs=act_in[:, c, :, 512 * n:512 * (n + 1)],
                    start=(c == 0),
                    stop=(c == C - 1),
                    perf_mode=DR,
                )
        for n in range(NT):
            nc.vector.tensor_scalar(
                out=act_out[:, mt // 2, mt % 2, 512 * n:512 * (n + 1)],
                in0=pss[n][:],
                scalar1=thr_sb[:, mt:mt + 1],
                scalar2=None,
                op0=mybir.AluOpType.is_ge,
            )


def build_program(bc=BC, dump_acts=False):
    """Build the per-core Bass/Tile program (SPMD; identical on all cores)."""
    NT = bc // 512
    BT = bc // 128
    DR = mybir.MatmulPerfMode.DoubleRow

    nc = bacc.Bacc(None, target_bir_lowering=False, debug=False)
    dbg = {}
    if dump_acts:
        for nm in ("act1d", "act2d", "act3d", "act4d"):
            cdim = 3 if nm == "act1d" else 16
            dbg[nm] = nc.dram_tensor(
                nm, [128, cdim, 2, bc], FP8, kind="ExternalOutput")
        dbg["h4d"] = nc.dram_tensor("h4d", [16, bc], FP32, kind="ExternalOutput")

    xt = nc.dram_tensor("xt", [IND, bc], BF16, kind="ExternalInput")
    w1 = nc.dram_tensor("w1dr", [32, 128, 3, 2, 128], FP8, kind="ExternalInput")
    w2 = nc.dram_tensor("w2dr", [32, 128, 16, 2, 128], FP8, kind="ExternalInput")
    w3 = nc.dram_tensor("w3dr", [32, 128, 16, 2, 128], FP8, kind="ExternalInput")
    w4 = nc.dram_tensor("w4dr", [128, 16, 2, 16], FP8, kind="ExternalInput")
    thrs = nc.dram_tensor("thrs", [128, 3, 32], FP32, kind="ExternalInput")
    c4 = nc.dram_tensor("c4", [16, 2], FP32, kind="ExternalInput")
    out = nc.dram_tensor("out", [128, bc // 128, OUT], FP32,
                         kind="ExternalOutput")

    with tile.TileContext(nc) as tc, ExitStack() as ctx:
        consts = ctx.enter_context(tc.tile_pool(name="consts", bufs=1))
        xpool = ctx.enter_context(tc.tile_pool(name="xpool", bufs=2))
        a1pool = ctx.enter_context(tc.tile_pool(name="a1pool", bufs=1))
        apool = ctx.enter_context(
            tc.tile_pool(name="apool", bufs=3 if dump_acts else 2))
        wpool = ctx.enter_context(tc.tile_pool(name="wpool", bufs=4))
        smpool = ctx.enter_context(tc.tile_pool(name="smpool", bufs=3))
        psum_pool = ctx.enter_context(
            tc.tile_pool(name="psum", bufs=8, space="PSUM"))

        thrs_sb = consts.tile([128, 3, 32], FP32, tag="thrs")
        c4_sb = consts.tile([16, 2], FP32, tag="c4")
        w4_sb = consts.tile([128, 16, 2, 16], FP8, tag="w4")
        ident = consts.tile([128, 128], FP32, tag="ident")
        h4 = consts.tile([16, bc], FP32, tag="h4")
        out_sb = consts.tile([128, BT, OUT], FP32, tag="outsb")
        thr1_sb = thrs_sb[:, 0, :]
        thr2_sb = thrs_sb[:, 1, :]
        thr3_sb = thrs_sb[:, 2, :]

        # ---- binarize x: act1 = [x >= 0] in {0,1} fp8, feature-major.
        # HWDGE descriptor-gen is ~0.65us *serial* per dma_start, so x goes
        # as TWO batched strided DMAs (one per ring), emitted before all
        # const DMAs. xa carries k-chunk c0 (the layer-1 critical path),
        # xb carries c1+c2; binarize is one DVE op per k-chunk.
        act1 = a1pool.tile([128, 3, 2, bc], FP8, tag="act1")
        xa = xpool.tile([128, 2, bc], BF16, tag="xa", bufs=1)
        xb = xpool.tile([128, 4, bc], BF16, tag="xb", bufs=1)
        nc.sync.dma_start(
            out=xa[:], in_=xt[0:256, :].rearrange("(i p) n -> p i n", p=128))
        nc.scalar.dma_start(
            out=xb[:], in_=xt[256:768, :].rearrange("(i p) n -> p i n", p=128))

        # consts follow x on the rings; their data is needed much later
        nc.scalar.dma_start(out=thrs_sb[:], in_=thrs[:])
        nc.scalar.dma_start(out=c4_sb[:], in_=c4[:])
        nc.scalar.dma_start(out=w4_sb[:], in_=w4[:])
        make_identity(nc, ident[:])

        # PE warm-up: the HAM clock gate needs ~3.4us of sustained matmul
        # activity to lift the PE from 1.2 to 2.4 GHz. The PE would otherwise
        # idle for ~9us waiting on the x DMA and start the real stream cold.
        # ~36 garbage DR matmuls (memset operands, never-read psum) span the
        # wait so layer 1 opens at full clock.
        warm = consts.tile([128, 2, 512], FP8, tag="warm")
        nc.gpsimd.memset(warm[:], 0.0)
        wps = psum_pool.tile([128, 512], FP32, tag="psum", name="warmps")
        for _ in range(36):
            nc.tensor.matmul(
                wps[:], lhsT=warm[:, :, 0:128], rhs=warm[:],
                start=True, stop=True, perf_mode=DR)

        nc.vector.tensor_scalar(
            out=act1[:, 0, :, :], in0=xa[:], scalar1=0.0, scalar2=None,
            op0=mybir.AluOpType.is_ge)
        for c in (1, 2):
            nc.vector.tensor_scalar(
                out=act1[:, c, :, :], in0=xb[:, 2 * (c - 1):2 * c, :],
                scalar1=0.0, scalar2=None, op0=mybir.AluOpType.is_ge)

        # ---- layers 1-3 ----
        act2 = apool.tile([128, 16, 2, bc], FP8, tag="actbig")
        _layer_fwd(nc, wpool, psum_pool, act1, 3, w1, thr1_sb, act2, 32, bc,
                   dma_engine=nc.scalar)
        act3 = apool.tile([128, 16, 2, bc], FP8, tag="actbig")
        _layer_fwd(nc, wpool, psum_pool, act2, 16, w2, thr2_sb, act3, 32, bc)
        act4 = apool.tile([128, 16, 2, bc], FP8, tag="actbig")
        _layer_fwd(nc, wpool, psum_pool, act3, 16, w3, thr3_sb, act4, 32, bc)

        # ---- layer 4: logits (M padded 10->16), affine folds BN+rowsum.
        # Softmax is phased to avoid ACT table thrash (Exp/Ln swaps).
        sh = smpool.tile([128, BT, OUT], FP32, tag="sh", bufs=1)
        se = smpool.tile([128, BT], FP32, tag="se", bufs=1)
        ls = smpool.tile([128, BT], FP32, tag="ls", bufs=1)

        def _l4_softmax_head(g):
            # transpose group g's batch tiles + max/shift on DVE; runs one
            # n-group behind the L4 matmuls so the PE never stalls on it
            for bt in range(4 * g, 4 * g + 4):
                tp = psum_pool.tile([128, OUT], FP32, tag="psum",
                                    name=f"tp{bt}")
                nc.tensor.transpose(
                    tp[:], h4[0:OUT, 128 * bt:128 * (bt + 1)],
                    ident[0:OUT, 0:OUT])
                mx = smpool.tile([128, 1], FP32, tag="mx", name=f"mx{bt}")
                nc.vector.reduce_max(mx[:], tp[:], axis=mybir.AxisListType.X)
                nc.vector.tensor_scalar(
                    out=sh[:, bt, :], in0=tp[:], scalar1=mx[:], scalar2=None,
                    op0=mybir.AluOpType.subtract)
        for n in range(NT):
            ps4 = psum_pool.tile([16, 512], FP32, tag="psum", name=f"ps4_{n}")
            for c in range(16):
                nc.tensor.matmul(
                    ps4[:],
                    lhsT=w4_sb[:, c, :, :],
                    rhs=act4[:, c, :, 512 * n:512 * (n + 1)],
                    start=(c == 0),
                    stop=(c == 15),
                    perf_mode=DR,
                )
            # affine on the (idle) scalar engine: out = in*scale + bias
            nc.scalar.activation(
                h4[:, 512 * n:512 * (n + 1)], ps4[:],
                mybir.ActivationFunctionType.Identity,
                bias=c4_sb[:, 1:2], scale=c4_sb[:, 0:1],
            )
        for g in range(NT):
            _l4_softmax_head(g)
        ex = smpool.tile([128, BT, OUT], FP32, tag="ex", bufs=1)
        for bt in range(BT):  # all Exp together: one ACT table load
            nc.scalar.activation(
                ex[:, bt, :], sh[:, bt, :], mybir.ActivationFunctionType.Exp,
                accum_out=se[:, bt:bt + 1])
        nc.scalar.activation(  # single Ln over all batch tiles
            ls[:], se[:], mybir.ActivationFunctionType.Ln)
        for bt in range(BT):
            nc.vector.tensor_scalar(
                out=out_sb[:, bt, :], in0=sh[:, bt, :],
                scalar1=ls[:, bt:bt + 1], scalar2=None,
                op0=mybir.AluOpType.subtract)

        # out dram is [128, BT, OUT] (partition-major, fully contiguous DMA);
        # the host reassembles batch order with a free transpose.
        nc.sync.dma_start(out=out[:], in_=out_sb[:])

        if dump_acts:
            nc.sync.dma_start(out=dbg["act1d"][:], in_=act1[:])
            nc.sync.dma_start(out=dbg["act2d"][:], in_=act2[:])
            nc.sync.dma_start(out=dbg["act3d"][:], in_=act3[:])
            nc.sync.dma_start(out=dbg["act4d"][:], in_=act4[:])
            nc.sync.dma_start(out=dbg["h4d"][:], in_=h4[:])

    nc.compile()
    return nc


# --------------------------------------------------------------------------
# Host-side preparation
# --------------------------------------------------------------------------

def _pack_w_dr(ws_t):
    """[Fin, Fout] {-1,+1} -> [Mt, 128, C, 2, 128] fp8 DoubleRow layout.

    wdr[mt, ki, c, ko, mi] = ws_t[256*c + 128*ko + ki, 128*mt + mi]
    """
    fin, fout = ws_t.shape
    C, Mt = fin // 256, fout // 128
    w = ws_t.reshape(C, 2, 128, Mt, 128).transpose(3, 2, 0, 1, 4)
    return np.ascontiguousarray(w).astype(NP_FP8)


def prepare_consts(inputs):
    """Fold sign(w), BN, bias and the 0/1-activation rowsum correction.

    The device computes, per layer, a_dev = [mmA~ >= thr] where
    mmA~ = W~sign @ a_dev_prev over {0,1} activations. Negative BN scales
    (alpha <= 0) are handled exactly by tracking a per-neuron flip bit
    (a_true = 1 - a_dev) that folds into the *next* layer's weight signs:
    with s~ = s * (1-2*flip_in), mm_full = 2*(s~ @ a_dev) - rowsum(s~)
    holds for any flip pattern. Thresholds use integer snapping (mmA is
    always an integer), making the device comparison tie-free/exact.
    """
    consts = {}
    flip_in = np.zeros(IND)  # input layer: a_dev = [x >= 0] = ste_sign, exact
    for i in (1, 2, 3, 4):
        w = np.asarray(inputs[f"w{i}"]).astype(np.float64)
        b = np.asarray(inputs[f"b{i}"]).astype(np.float64)
        g = np.asarray(inputs[f"g{i}"]).astype(np.float64)
        be = np.asarray(inputs[f"be{i}"]).astype(np.float64)
        m = np.asarray(inputs[f"m{i}"]).astype(np.float64)
        v = np.asarray(inputs[f"v{i}"]).astype(np.float64)
        ws = np.where(w >= 0, 1.0, -1.0) * (1.0 - 2.0 * flip_in)  # [fo, fi]
        rowsum = ws.sum(axis=1)                                   # [fo]
        alpha = g / np.sqrt(v + EPS)
        if i < 4:
            # BN(mm_full + b) >= 0 with mm_full = 2*mmA - rowsum:
            #   alpha > 0:  a_true = [mmA >= u],  u = (m-b-be/a+rowsum)/2
            #   alpha < 0:  a_true = [mmA <= u] = 1 - [mmA >= floor(u)+1]
            #   alpha == 0: BN = be, constant sign
            u = (m - b - be / alpha_safe(alpha) + rowsum) / 2.0
            pos = alpha > 0
            thr = np.where(pos, np.ceil(u), np.floor(u) + 1.0)
            zero = alpha == 0
            if zero.any():
                # constant: a_true = [be >= 0]; force a_dev accordingly
                thr = np.where(zero & (be >= 0), -1e30, thr)
                thr = np.where(zero & (be < 0), 1e30, thr)
                pos = pos | zero
            flip_in = (~pos).astype(np.float64)
            consts.setdefault("_thrs", []).append(
                thr.reshape(32, 128).T.astype(np.float32))
            consts[f"w{i}dr"] = _pack_w_dr(ws.T)
        else:
            # logits = mmA*(2*alpha) + ((b - m - rowsum)*alpha + be), pad to 16
            scale = 2.0 * alpha
            beta = (b - m - rowsum) * alpha + be
            c4 = np.zeros((16, 2), np.float32)
            c4[:10, 0] = scale.astype(np.float32)
            c4[:10, 1] = beta.astype(np.float32)
            consts["c4"] = c4
            ws_t_pad = np.zeros((HID, 16), np.float64)
            ws_t_pad[:, :10] = ws.T
            # w4dr[ki, c, ko, m] = ws_t_pad[256*c + 128*ko + ki, m]
            w4 = ws_t_pad.reshape(16, 2, 128, 16).transpose(2, 0, 1, 3)
            consts["w4dr"] = np.ascontiguousarray(w4).astype(NP_FP8)
    consts["thrs"] = np.ascontiguousarray(
        np.stack(consts.pop("_thrs"), axis=1))  # [128, 3, 32]
    return consts


def alpha_safe(a):
    return np.where(a == 0, 1.0, a)


_PROG_CACHE = {}


def _get_program(bc=BC):
    if bc not in _PROG_CACHE:
        _PROG_CACHE[bc] = build_program(bc)
    return _PROG_CACHE[bc]


def kernel(**inputs):
    global LAST_RESULTS
    x = np.asarray(inputs["x"], np.float32)
    assert x.shape == (B, 784)
    consts = prepare_consts(inputs)
    # bf16 halves the x DMA; sign(x) is unaffected (bf16 RNE preserves sign
    # for all float32 normals, and -0.0 >= 0 is true in both).
    xt_full = np.ascontiguousarray(x[:, :IND].T).astype(NP_BF16)  # [768, B]

    nc = _get_program(BC)
    in_maps = []
    for c in range(N_CORES):
        m = {"xt": np.ascontiguousarray(xt_full[:, c * BC:(c + 1) * BC])}
        m.update(consts)
        in_maps.append(m)

    res = run_bass_kernel_spmd(
        nc, in_maps, core_ids=list(range(N_CORES)), trace=TRACE,
        **TRACE_KWARGS)
    LAST_RESULTS = res
    # device out is [128, BT, 10] partition-major; restore batch order
    outs = [np.ascontiguousarray(r["out"].transpose(1, 0, 2).reshape(BC, OUT))
            for r in res.results]
    return np.concatenate(outs, axis=0)
